# revision 31
# baseline (speedup 1.0000x reference)
"""Trainium2 Bass kernel for nn_NeuralAdditiveModel_81930796138712.

Reference computation (B=8192, F=256, H1=64, H2=32, C=10):
    h1 = relu(x[:, :, None] * W1[:, 0, :] + b1)        # [B, F, H1]
    h2 = relu(einsum('bfh,fho->bfo', h1, W2) + b2)     # [B, F, H2]
    out = einsum('bfo,foc->bfc', h2, Wout).sum(1) + bias

Key structural fact: in this problem b1 == 0 and b2 == 0 (the reference's
setup_inputs hardcodes jnp.zeros for them).  A 1-input MLP with zero biases
is positively homogeneous in its scalar input, so each per-feature net is
exactly a two-piece linear map through the origin:

    relu(s*w) = max(s,0)*relu(w) + max(-s,0)*relu(-w)          (exact)

Folding that identity through both relu layers collapses the whole model to

    out = x @ U + relu(-x) @ (U + V) + bias

with U, V [F, C] precomputed on the host from W1/W2/Wout (float64, exact).
The device kernel is then a thin, purely memory-bound matmul pair.

Sharding: data-parallel over the batch (8192 -> 8 x 1024), weights
replicated; per-core output slices are disjoint so no collectives at all.

Device layout per core (batch n = 1024):
    xt  [256, n]   f32r  x-slice transposed on host (features on partitions)
    s   [128, 41]  f32r  stationary stack [U0|U1|(U+V)0|(U+V)1|bias-col]
    y   [10, n]    f32   out-slice transposed (classes on partitions)

Per batch tile: relu(-x) on the vector engine, 4 accumulating float32r
matmuls (K-chunks of 128 features), then a scalar-engine Identity+bias
PSUM evacuation; dummy bf16 matmuls warm the PE clock while the first x
DMA is in flight.  Host does the x transpose / y gather-transpose.
"""

import sys

if "/opt/trn_rl_repo" not in sys.path:
    sys.path.insert(0, "/opt/trn_rl_repo")

import numpy as np

B, F, H1, H2, C = 8192, 256, 64, 32, 10
N_CORES = 8
B_LOC = B // N_CORES  # 1024

_NC_CACHE = {}
LAST_RESULTS = None  # BassKernelResults of the most recent device run

# Kernel-structure knobs (see _build_nc docstring).
TILES = (256, 256, 256, 256)
WARMUP_MMS = 5
HALF_GRAIN = True


def _build_nc(run_tag: str = "", *, tiles=(512, 512), warmup_mms=5, half_grain=True):
    """Build the per-core Bass program.

    out = x @ U + relu(-x) @ (U+V) + bias, with the stationary [U|U+V]
    stack resident in SBUF and x streamed through the PE as float32r.

    tiles: batch-tile widths (sum == B_LOC, each <= 512; >= 256 keeps
        float32r streaming at full rate).  A smaller LAST tile shortens the
        critical tail (DVE x- pass -> V-matmuls -> epilogue -> y store).
    warmup_mms: dummy matmuls on a zeroed scratch tile issued while the x
        DMA is in flight — keeps the PE clock (HAM gate) ramped so the
        real matmuls run at full rate.
    half_grain: compute the x- pass and V-matmuls per feature half so the
        PE starts on half the data as soon as the DVE produces it.
    """
    import concourse.mybir as mybir
    from concourse import bacc
    from concourse.alu_op_type import AluOpType as AluOp
    from concourse.tile import TileContext

    assert sum(tiles) == B_LOC
    assert half_grain, "zeros tile is sized for per-half passes"
    f32 = mybir.dt.float32
    f32r = mybir.dt.float32r
    # Bacc (not raw Bass): its compile() runs split_sync_waits /
    # move_matmul_waits_to_ldweights, which legalize the 1-wait-per-
    # instruction TRN2 constraint that walrus enforces.
    nc = bacc.Bacc(name="nam_collapsed" + run_tag)

    xt = nc.dram_tensor("xt", [F, B_LOC], f32r, kind="ExternalInput")
    # cols 0:40 = stationary K-chunks; col 40 = output bias
    # (partitions 0..C-1).  Packed together to keep the DMA count low.
    s = nc.dram_tensor("s", [128, 41], f32r, kind="ExternalInput")
    y = nc.dram_tensor("y", [C, B_LOC], f32, kind="ExternalOutput")

    n_bt = len(tiles)
    offs = [sum(tiles[:i]) for i in range(n_bt + 1)]
    bt_max = max(tiles)

    with TileContext(nc) as tc:
        with (
            tc.tile_pool(name="consts", bufs=1) as consts,
            tc.tile_pool(name="sb", bufs=n_bt) as sb,
            tc.tile_pool(name="osb", bufs=1) as osb,
            tc.tile_pool(name="ps", bufs=min(n_bt, 4), space="PSUM") as ps,
            tc.tile_pool(name="warm", bufs=1, space="PSUM") as warm_pool,
        ):
            # x loads first: they own the DMA engines; everything else
            # (s load via SWDGE, memsets via Pool) runs in their shadow.
            xts = []
            for t, bt in enumerate(tiles):
                sl = slice(offs[t], offs[t + 1])
                # One DMA per batch tile bringing BOTH feature halves:
                # cols 0:bt = features 0..127, bt:2*bt = features 128..255
                x_t = sb.tile([128, 2 * bt], f32r, tag="x")
                nc.sync.dma_start(
                    x_t[:, :].rearrange("p (a b) -> p a b", a=2),
                    xt[:, sl].rearrange("(a p) b -> p a b", p=128),
                )
                xts.append(x_t)

            s_sb = consts.tile([128, 41], f32r)
            # SWDGE (gpsimd) path: separate DGE dispatch device from the
            # HWDGE x loads.
            nc.gpsimd.dma_start(s_sb[:, :], s[:, :])
            b_sb = s_sb[0:C, 40:41].bitcast(f32)

            o_sb = osb.tile([C, B_LOC], f32)

            if warmup_mms:
                # Keep the PE busy while the x DMA is in flight so the HAM
                # clock gate is released by the time real matmuls issue.
                # bf16 scratch: memset supports it and it streams 1 cyc/row.
                wsrc = consts.tile([128, 512], mybir.dt.bfloat16)
                nc.gpsimd.memset(wsrc[:, :], 0)
                wps = warm_pool.tile([C, 512], f32)
                for _ in range(warmup_mms):
                    nc.tensor.matmul(
                        wps[:, :], wsrc[:, 0:10], wsrc[:, :],
                        start=True, stop=True,
                    )

            zeros = consts.tile([128, bt_max], f32, tag="zeros")
            nc.gpsimd.memset(zeros[:, :], 0)

            for t, bt in enumerate(tiles):
                sl = slice(offs[t], offs[t + 1])
                x_t = xts[t]
                pt = ps.tile([C, bt_max], f32, tag="pt")

                # U-matmuls on x need no pointwise prep at all.
                nc.tensor.matmul(pt[:, 0:bt], s_sb[:, 0:10], x_t[:, 0:bt], start=True, stop=False)
                nc.tensor.matmul(pt[:, 0:bt], s_sb[:, 10:20], x_t[:, bt : 2 * bt], start=False, stop=False)

                # x- = max(-x, 0) = (x * -1) max 0 on the vector engine,
                # then the V-matmuls; per feature half when half_grain so
                # each V-matmul only depends on its own half of the DVE pass.
                n_t = sb.tile([128, 2 * bt], f32r, tag="n")
                if half_grain:
                    halves = [(0, bt), (bt, 2 * bt)]
                else:
                    halves = [(0, 2 * bt)]
                for h0, h1 in halves:
                    nc.vector.scalar_tensor_tensor(
                        n_t[:, h0:h1], x_t[:, h0:h1], -1.0,
                        zeros[:, 0 : h1 - h0],
                        op0=AluOp.mult, op1=AluOp.max,
                    )
                    for k0 in range(h0, h1, bt):
                        nc.tensor.matmul(
                            pt[:, 0:bt], s_sb[:, 20 + (k0 // bt) * 10 : 30 + (k0 // bt) * 10],
                            n_t[:, k0 : k0 + bt],
                            start=False, stop=(k0 + bt == 2 * bt),
                        )

                # PSUM -> SBUF evacuation fused with the output-bias add:
                # out = Identity(psum * 1.0 + bias[partition])
                nc.scalar.activation(
                    o_sb[:, sl],
                    pt[:, 0:bt],
                    mybir.ActivationFunctionType.Identity,
                    bias=b_sb,
                    scale=1.0,
                )
                nc.sync.dma_start(y[:, sl], o_sb[:, sl])

    nc.compile()
    return nc


def _collapse_weights(W1, W2, Wout):
    """Fold the zero-bias per-feature MLPs into x/|x| coefficients (exact)."""
    W1f = W1.astype(np.float64)[:, 0, :]        # [F, H1]
    W2f = W2.astype(np.float64)                 # [F, H1, H2]
    Wof = Wout.astype(np.float64)               # [F, H2, C]
    P = np.einsum("fh,fho->fo", np.maximum(W1f, 0), W2f)
    Q = np.einsum("fh,fho->fo", np.maximum(-W1f, 0), W2f)
    U = np.einsum("fo,foc->fc", np.maximum(P, 0), Wof)   # coefficient of x+
    V = np.einsum("fo,foc->fc", np.maximum(Q, 0), Wof)   # coefficient of x-
    return U.astype(np.float32), V.astype(np.float32)


def _reference_host(x, W1, b1, W2, b2, Wout, bias):
    """Honest numpy fallback (never taken for this problem's inputs)."""
    h1 = np.maximum(x[:, :, None] * W1[:, 0, :][None] + b1[None], 0.0)
    h2 = np.maximum(np.einsum("bfh,fho->bfo", h1, W2) + b2[None], 0.0)
    return np.einsum("bfo,foc->bc", h2, Wout) + bias


def kernel(x, W1, b1, W2, b2, Wout, bias):
    global LAST_RESULTS
    x = np.asarray(x, np.float32)
    bias = np.asarray(bias, np.float32)

    if np.any(np.asarray(b1)) or np.any(np.asarray(b2)):
        # Zero-bias collapse does not apply; compute exactly on host.
        return _reference_host(
            np.asarray(x, np.float64), np.asarray(W1, np.float64),
            np.asarray(b1, np.float64), np.asarray(W2, np.float64),
            np.asarray(b2, np.float64), np.asarray(Wout, np.float64),
            np.asarray(bias, np.float64),
        ).astype(np.float32)

    U, V = _collapse_weights(np.asarray(W1), np.asarray(W2), np.asarray(Wout))

    # out = x @ U + relu(-x) @ (U + V):
    #   x>0: x@U;  x<0: x@U + (-x)@(U+V) = (-x)@V = x- @ V.   (exact)
    first, second = U, U + V

    # Stationary stack, SBUF layout [128 partitions, 41 cols]:
    # cols 0:10/10:20 = first K-chunks, 20:30/30:40 = second K-chunks,
    # col 40 = output bias on partitions 0..C-1.
    bias_col = np.zeros((128, 1), np.float32)
    bias_col[:C, 0] = bias
    s_host = np.concatenate(
        [first[:128], first[128:], second[:128], second[128:], bias_col], axis=1
    ).astype(np.float32)
    s_host = np.ascontiguousarray(s_host)

    xt = np.ascontiguousarray(x.T)              # [F, B]

    from concourse.bass_utils import run_bass_kernel_spmd

    if "nc" not in _NC_CACHE:
        _NC_CACHE["nc"] = _build_nc(
            tiles=TILES, warmup_mms=WARMUP_MMS, half_grain=HALF_GRAIN
        )
    nc = _NC_CACHE["nc"]

    in_maps = [
        {
            "xt": np.ascontiguousarray(xt[:, c * B_LOC : (c + 1) * B_LOC]),
            "s": s_host,
        }
        for c in range(N_CORES)
    ]

    LAST_RESULTS = run_bass_kernel_spmd(nc, in_maps, core_ids=list(range(N_CORES)))
    out = np.concatenate(
        [LAST_RESULTS.results[c]["y"].T for c in range(N_CORES)], axis=0
    )
    return np.ascontiguousarray(out.astype(np.float32))
